# revision 21
# baseline (speedup 1.0000x reference)
"""AttentionBlock (GroupNorm + 1x1-conv self-attention + proj + residual) on 8 TRN2 cores.

Strategy: data-parallel over batch (16 samples -> 2 per core). Per sample, the
whole block runs out of SBUF with two host-side weight fusions that exploit the
bilinearity of attention:
  - logits = (Wq xn)^T (Wk xn) = xn^T (Wq^T Wk) xn, so one fused weight
    Wqk = Wk^T Wq gives z = Wqk xn and logitsT[m,n] = xn[:,m] . z[:,n]
    (the k-bias term is constant along the softmax axis and drops; the q-bias
    term is restored as a per-partition bias in the exp when nonzero).
  - proj(V E) = (Wp Wv) xn E, so Wpv = Wp Wv gives pvT[m,o] = xn[:,m].WpvT[:,o]
    (computed directly transposed by making xn the stationary operand) and
    PU = pvT^T E needs no separate v / attn-out / proj matmuls. The v-bias
    contributes (Wp vb) * denom[n], which after normalization is a constant
    per-channel shift folded into the proj bias.
  This cuts the per-sample matmul count from 272 to 208.
  - GroupNorm(8 groups): per-partition bn_stats on DVE, cross-partition group
    sums via one masked full-K ones-matmul (fp32r; M=1 / partial-K matmuls
    abort on this hw, so mask + full 128-row ones instead).
  - softmax: exp on ScalarE with the 1/sqrt(c) scale folded in (max-subtraction
    skipped: scaled logits are ~N(0,1)), denominator via ones-matmul rows (every
    psum row = denom, no cross-partition broadcast), reciprocal_approx_fast,
    normalization deferred past the (fused) AV+proj matmul into the final
    residual-add pass on VectorE.
  - Data-path matmuls in fp16 (chain error ~1e-4, measured on hw).
"""

import os
import sys

for _p in ("/root/.axon_site", "/root/.axon_site/_ro/trn_rl_repo", "/opt/trn_rl_repo"):
    if os.path.isdir(_p) and _p not in sys.path:
        sys.path.append(_p)

import numpy as np

import concourse.bass as bass
import concourse.tile as tile
from concourse import bacc, mybir
from concourse.bass_utils import run_bass_kernel_spmd

N_CORES = 8
B, C, H, W = 16, 512, 32, 32
HW = H * W                  # 1024 spatial positions
BPC = B // N_CORES          # samples per core
CO = C // 128               # 4 channel tiles
MT = HW // 128              # 8 spatial (m) tiles
NCH = HW // 512             # 2 free-dim chunks of 512
GROUPS = 8
EPS = 1e-5
SCALE = float(C) ** -0.5
N_WARM = 40                 # dummy matmuls bridging the PE through startup

F32 = mybir.dt.float32
F32R = mybir.dt.float32r
F16 = mybir.dt.float16
AF = mybir.ActivationFunctionType

TRACE = False               # test.py sets kernel.TRACE = True for NTFF timing

_CACHE: dict = {}


def _build(flags):
    has_qb, has_pb, has_gamma, has_beta = flags
    nc = bacc.Bacc(trn_type="TRN2", target_bir_lowering=False, debug=False,
                   num_devices=N_CORES)

    xs = nc.dram_tensor("xs", [BPC, C, HW], F32, kind="ExternalInput").ap()
    wqk = nc.dram_tensor("wqk", [C, C], F16, kind="ExternalInput").ap()   # (Wk^T Wq)^T
    wpv = nc.dram_tensor("wpv", [C, C], F16, kind="ExternalInput").ap()   # (Wp Wv)^T
    gam = nc.dram_tensor("gam", [128, CO], F32, kind="ExternalInput").ap()
    bet = nc.dram_tensor("bet", [128, CO], F32, kind="ExternalInput").ap()
    wf = nc.dram_tensor("wf", [C, 1], F16, kind="ExternalInput").ap()     # Wk^T bq
    pbt = nc.dram_tensor("pbt", [128, CO], F32, kind="ExternalInput").ap()
    out = nc.dram_tensor("out", [BPC, C, HW], F32, kind="ExternalOutput").ap()

    with tile.TileContext(nc) as tc:
        with (
            tc.tile_pool(name="wpool", bufs=1) as wpool,
            tc.tile_pool(name="xfp", bufs=2) as xfp,
            tc.tile_pool(name="xnp", bufs=2) as xnp,
            tc.tile_pool(name="zpv", bufs=2) as zpv,
            tc.tile_pool(name="fin", bufs=4) as finp,
            tc.tile_pool(name="hpool", bufs=2) as hpool,
            tc.tile_pool(name="small", bufs=4) as small,
            tc.tile_pool(name="mmps", bufs=(6 if has_qb else 7), space="PSUM") as mmps,
            tc.tile_pool(name="dps", bufs=1, space="PSUM") as dps,
        ):
            # ---- x(s0) first so groupnorm starts ASAP; weights overlap ----
            xs_t = [xs[s].rearrange("(co p) n -> p co n", p=128) for s in range(BPC)]
            xfs = []
            for s in range(BPC):
                xfs.append(xfp.tile([128, CO, HW], F32, tag="xf", name=f"xf{s}"))
            for co in range(CO):
                nc.sync.dma_start(xfs[0][:, co], xs_t[0][:, co])

            onesf = wpool.tile([128, 512], F32, tag="onesf")
            nc.vector.memset(onesf, 1.0)
            ones_r = wpool.tile([128, 128], F32R, tag="onesr")
            nc.vector.tensor_copy(ones_r, onesf[:, 0:128])
            ones16 = wpool.tile([128, 512], F16, tag="ones16")
            nc.vector.tensor_copy(ones16, onesf)

            # PE warmup: harmless matmuls bridge the HAM clock gate through the
            # DMA + groupnorm startup so the first real matmuls run at 2.4 GHz.
            def warmup(n, nm):
                warm_ps = mmps.tile([128, 512], F32, tag="mm", name=f"warm{nm}")
                for i in range(n):
                    nc.tensor.matmul(warm_ps, ones16[:, 0:128], ones16,
                                     start=(i == 0), stop=(i == n - 1))
            warmup(20, "a")

            wqk_sb = wpool.tile([128, CO, C], F16, tag="wqk")
            nc.sync.dma_start(wqk_sb, wqk.rearrange("(ci p) o -> p ci o", p=128))
            wpv_sb = wpool.tile([128, CO, C], F16, tag="wpv")
            nc.sync.dma_start(wpv_sb, wpv.rearrange("(ci p) o -> p ci o", p=128))

            gam_sb = wpool.tile([128, CO], F32, tag="gam")
            nc.sync.dma_start(gam_sb, gam)
            bet_sb = wpool.tile([128, CO], F32, tag="bet")
            nc.sync.dma_start(bet_sb, bet)
            pbt_sb = wpool.tile([128, CO], F32, tag="pbt")
            nc.sync.dma_start(pbt_sb, pbt)
            if has_qb:
                wf_sb = wpool.tile([128, CO, 1], F16, tag="wf")
                nc.sync.dma_start(wf_sb, wf.rearrange("(ci p) o -> p ci o", p=128))

            for s in range(1, BPC):
                for co in range(CO):
                    nc.sync.dma_start(xfs[s][:, co], xs_t[s][:, co])

            eps_sb = wpool.tile([1, 1], F32, tag="eps")
            nc.vector.memset(eps_sb, EPS)
            # per-partition group-half masks: lo = partitions 0-63, hi = 64-127
            mask_lo = wpool.tile([128, 1], F32, tag="mlo")
            nc.vector.memset(mask_lo, 0.0)
            nc.vector.memset(mask_lo[0:64], 1.0)
            mask_hi = wpool.tile([128, 1], F32, tag="mhi")
            nc.vector.tensor_sub(mask_hi, onesf[:, 0:1], mask_lo)

            # ================= group norm (emitted per sample) =========
            xns = [None] * BPC
            sq_scratch = wpool.tile([128, 1024], F16, tag="sqscr")

            def groupnorm(s):
                xf = xfs[s]
                # sp_wide[:, 4co:4co+4] = [m*lo, E2*lo, m*hi, E2*hi]: a single
                # full-K ones-matmul yields every group sum on psum row 0.
                # Even co tiles compute stats on ScalarE (activation accum_out),
                # odd ones on VectorE (bn_stats) - halves the serial stats time.
                sp_wide = small.tile([128, CO * 4], F32R, tag="spw")
                for co in range(CO):
                    sp = small.tile([128, 2], F32, tag="sp")
                    if co % 2 == 0:
                        nc.scalar.activation(out=sq_scratch, in_=xf[:, co],
                                             func=AF.Copy, scale=1.0 / 1024.0,
                                             accum_out=sp[:, 0:1])
                        nc.scalar.activation(out=sq_scratch, in_=xf[:, co],
                                             func=AF.Square, scale=1.0 / 32.0,
                                             accum_out=sp[:, 1:2])
                    else:
                        st = small.tile([128, 2, 6], F32, tag="bnst")
                        for ch in range(2):
                            nc.vector.bn_stats(st[:, ch], xf[:, co, ch * 512:(ch + 1) * 512])
                        mv = small.tile([128, 2], F32, tag="mv")
                        nc.vector.bn_aggr(mv, st)
                        # sp = [mean, var + mean^2] (= [mean, E[x^2]])
                        sq = small.tile([128, 1], F32, tag="sq")
                        nc.vector.tensor_mul(sq, mv[:, 0:1], mv[:, 0:1])
                        nc.vector.tensor_copy(sp[:, 0:1], mv[:, 0:1])
                        nc.vector.tensor_add(sp[:, 1:2], mv[:, 1:2], sq)
                    nc.vector.tensor_scalar_mul(sp_wide[:, 4 * co:4 * co + 2], sp, mask_lo)
                    nc.vector.tensor_scalar_mul(sp_wide[:, 4 * co + 2:4 * co + 4], sp, mask_hi)
                gst = mmps.tile([128, 16], F32, tag="mm", name=f"gst{s}")
                nc.tensor.matmul(gst, ones_r, sp_wide, start=True, stop=True)
                return gst

            def groupnorm_finish(s, gst):
                xf = xfs[s]
                gs = small.tile([1, 16], F32, tag="gs")
                nc.scalar.copy(gs, gst[0:1])
                # pk = [gm(8) | grstd(8)], one tile so one broadcast suffices
                pk = small.tile([1, 16], F32, tag="pk")
                nc.vector.tensor_scalar_mul(pk[:, 0:8], gs[:, 0:16:2], 1.0 / 64.0)
                gvar = small.tile([1, 8], F32, tag="gvar")
                nc.vector.tensor_tensor(gvar, pk[:, 0:8], gs[:, 0:16:2],
                                        mybir.AluOpType.mult)   # M * sum(m) = 64 M^2
                nc.vector.tensor_sub(gvar, gs[:, 1:16:2], gvar)  # sum(E2) - 64 M^2
                # sqrt((sum(E2) - 64 M^2)/64 + eps) = group std
                nc.scalar.activation(out=gvar, in_=gvar, func=AF.Sqrt,
                                     bias=eps_sb, scale=1.0 / 64.0)
                nc.vector.reciprocal(pk[:, 8:16], gvar)
                bc = small.tile([128, 16], F32, tag="bc")
                nc.gpsimd.partition_broadcast(bc, pk)
                # partitions 0-63 take even groups, 64-127 odd (blend by mask:
                # partition_broadcast corrupts base-64 out slices on hw)
                aA = small.tile([128, CO], F32, tag="aA")
                bM = small.tile([128, CO], F32, tag="bM")
                for dst, base in ((bM, 0), (aA, 8)):
                    od = small.tile([128, CO], F32, tag="bcod")
                    nc.vector.tensor_sub(od, bc[:, base + 1:base + 8:2],
                                         bc[:, base + 0:base + 8:2])
                    nc.vector.scalar_tensor_tensor(
                        out=dst, in0=od, scalar=mask_hi,
                        in1=bc[:, base + 0:base + 8:2],
                        op0=mybir.AluOpType.mult, op1=mybir.AluOpType.add)
                if has_gamma:
                    nc.vector.tensor_mul(aA, aA, gam_sb)
                bB = small.tile([128, CO], F32, tag="bB")
                nc.vector.tensor_mul(bB, bM, aA)        # M * A
                if has_beta:
                    nc.vector.tensor_sub(bB, bB, bet_sb)  # M*A - beta
                # xn = x*A - (M*A - beta) = (x - M)*A + beta
                xn = xnp.tile([128, CO, HW], F16, tag="xn", name=f"xn{s}")
                for co in range(CO):
                    nc.vector.tensor_scalar(
                        out=xn[:, co], in0=xf[:, co],
                        scalar1=aA[:, co:co + 1], scalar2=bB[:, co:co + 1],
                        op0=mybir.AluOpType.mult, op1=mybir.AluOpType.subtract)
                xns[s] = xn

            gst0 = groupnorm(0)
            warmup(16, "b")          # keep the PE hot while the stats chain runs
            groupnorm_finish(0, gst0)

            # ================= per-sample compute =================
            for s in range(BPC):
                xf, xn = xfs[s], xns[s]
                # ---- z = Wqk xn  ([c, n], fused q^T k weight) ----
                # ki outer: the moving operand stays fixed across the co MMs
                z_sb = zpv.tile([128, CO, HW], F16, tag="z")
                for nch in range(NCH):
                    psz = [mmps.tile([128, 512], F32, tag="mm", name=f"z{s}_{nch}_{co}")
                           for co in range(CO)]
                    for ki in range(CO):
                        for co in range(CO):
                            nc.tensor.matmul(
                                psz[co], wqk_sb[:, ki, 128 * co:128 * (co + 1)],
                                xn[:, ki, 512 * nch:512 * (nch + 1)],
                                start=(ki == 0), stop=(ki == CO - 1))
                    for co in range(CO):
                        nc.scalar.copy(z_sb[:, co, 512 * nch:512 * (nch + 1)], psz[co])
                if s + 1 < BPC:
                    _gst_next = groupnorm(s + 1)
                # ---- pvT[m, o] (fused proj@v, computed pre-transposed) ----
                pvT = zpv.tile([128, MT, C], F16, tag="pvT")
                if has_qb:
                    fq = zpv.tile([128, MT], F32, tag="fq")
                for mh in range(2):
                    pspv = [mmps.tile([128, 512], F32, tag="mm", name=f"pv{s}_{mh}_{i}")
                            for i in range(4)]
                    for ki in range(CO):
                        for i in range(4):
                            mt = 4 * mh + i
                            nc.tensor.matmul(
                                pspv[i], xn[:, ki, 128 * mt:128 * (mt + 1)],
                                wpv_sb[:, ki, :],
                                start=(ki == 0), stop=(ki == CO - 1))
                    for i in range(4):
                        nc.vector.tensor_copy(pvT[:, 4 * mh + i], pspv[i])
                    if has_qb:
                        # f[m] = (Wk^T bq) . xn[:, m], folded into the exp bias
                        # (4 single-column groups share one psum bank)
                        psf = dps.tile([128, 4], F32, tag="mmf", name=f"f{s}_{mh}")
                        for i in range(4):
                            mt = 4 * mh + i
                            for ki in range(CO):
                                nc.tensor.matmul(
                                    psf[:, i:i + 1], xn[:, ki, 128 * mt:128 * (mt + 1)],
                                    wf_sb[:, ki],
                                    start=(ki == 0), stop=(ki == CO - 1))
                        nc.scalar.activation(out=fq[:, 4 * mh:4 * mh + 4],
                                             in_=psf, func=AF.Copy, scale=SCALE)

                if s + 1 < BPC:
                    groupnorm_finish(s + 1, _gst_next)

                # ---- attention by n-half: logitsT = xn^T z, exp, PU = pvT^T E ----
                out_t = out[s].rearrange("(co p) n -> p co n", p=128)
                for h in range(NCH):
                    hs = slice(512 * h, 512 * (h + 1))
                    e_sb = hpool.tile([128, MT, 512], F16, tag="e")
                    for mh in range(2):
                        psl = [mmps.tile([128, 512], F32, tag="mm", name=f"l{s}_{h}_{mh}_{i}")
                               for i in range(4)]
                        for ki in range(CO):
                            for i in range(4):
                                mt = 4 * mh + i
                                nc.tensor.matmul(
                                    psl[i], xn[:, ki, 128 * mt:128 * (mt + 1)],
                                    z_sb[:, ki, hs],
                                    start=(ki == 0), stop=(ki == CO - 1))
                        for i in range(4):
                            mt = 4 * mh + i
                            if has_qb:
                                nc.scalar.activation(out=e_sb[:, mt], in_=psl[i],
                                                     func=AF.Exp, scale=SCALE,
                                                     bias=fq[:, mt:mt + 1])
                            else:
                                nc.scalar.activation(out=e_sb[:, mt], in_=psl[i],
                                                     func=AF.Exp, scale=SCALE)
                    # PU = pvT^T E (unnormalized); the moving operand e[mi]
                    # is shared by the 4 matmuls of each step
                    pspu = [mmps.tile([128, 512], F32, tag="mm", name=f"u{s}_{h}_{oo}")
                            for oo in range(CO)]
                    for mi in range(MT):
                        for oo in range(CO):
                            nc.tensor.matmul(pspu[oo],
                                             pvT[:, mi, 128 * oo:128 * (oo + 1)],
                                             e_sb[:, mi],
                                             start=(mi == 0), stop=(mi == MT - 1))
                    # denominator: pairwise e-tile adds on DVE, then one
                    # ones-matmul for the cross-partition sum (cheaper on the
                    # PE than 8 accumulating ones-matmuls)
                    t1 = hpool.tile([128, 4, 512], F16, tag="dt1")
                    for i in range(4):
                        nc.vector.tensor_add(t1[:, i], e_sb[:, 2 * i], e_sb[:, 2 * i + 1])
                    t2 = hpool.tile([128, 2, 512], F16, tag="dt2")
                    nc.vector.tensor_add(t2[:, 0], t1[:, 0], t1[:, 1])
                    nc.vector.tensor_add(t2[:, 1], t1[:, 2], t1[:, 3])
                    tS = hpool.tile([128, 512], F16, tag="dtS")
                    nc.vector.tensor_add(tS, t2[:, 0], t2[:, 1])
                    dsum = dps.tile([128, 512], F32, tag="dsum")
                    nc.tensor.matmul(dsum, ones16[:, 0:128], tS, start=True, stop=True)
                    rb = hpool.tile([128, 512], F32, tag="rb")
                    nc.vector.reciprocal_approx_fast(out=rb, in_=dsum)
                    for oo in range(CO):
                        t = finp.tile([128, 512], F32, tag="fin")
                        nc.vector.tensor_mul(t, pspu[oo], rb)
                        fo = finp.tile([128, 512], F32, tag="fo")
                        if has_pb:
                            nc.vector.scalar_tensor_tensor(
                                out=fo, in0=t,
                                scalar=pbt_sb[:, oo:oo + 1], in1=xf[:, oo, hs],
                                op0=mybir.AluOpType.add, op1=mybir.AluOpType.add)
                        else:
                            nc.vector.tensor_add(fo, t, xf[:, oo, hs])
                        nc.sync.dma_start(out_t[:, oo, hs], fo)

    nc.compile()
    return nc


def kernel(x, norm_w, norm_b, qkv_w, qkv_b, proj_w, proj_b):
    x = np.ascontiguousarray(np.asarray(x, dtype=np.float32).reshape(B, C, HW))
    norm_w = np.asarray(norm_w, dtype=np.float32)
    norm_b = np.asarray(norm_b, dtype=np.float32)
    qkv_w = np.asarray(qkv_w, dtype=np.float32)
    qkv_b = np.asarray(qkv_b, dtype=np.float32)
    proj_w = np.asarray(proj_w, dtype=np.float32)
    proj_b = np.asarray(proj_b, dtype=np.float32)

    flags = (
        bool(qkv_b[0:C].any()),
        True,   # proj-bias path also carries the folded v-bias; keep it on
        bool((norm_w != 1.0).any()), bool(norm_b.any()),
    )
    if flags not in _CACHE:
        _CACHE[flags] = _build(flags)
    nc = _CACHE[flags]

    Wq, Wk, Wv = qkv_w[0:C], qkv_w[C:2 * C], qkv_w[2 * C:]
    bq, bv = qkv_b[0:C], qkv_b[2 * C:]
    # z = (Wk^T Wq) xn ; device wants the transpose of that fused weight
    wqk_np = np.ascontiguousarray((Wq.T @ Wk).astype(np.float16))
    # pvT = xn^T (Wp Wv)^T
    wpv_np = np.ascontiguousarray((proj_w @ Wv).T.astype(np.float16))
    wf_np = np.ascontiguousarray((Wk.T @ bq).reshape(C, 1).astype(np.float16))
    # v-bias: proj(V + bv) = proj(V) + (Wp bv) * denom[n]; after the 1/denom
    # normalization that is a constant per-channel shift -> fold into proj_b
    pb_eff = proj_b + proj_w @ bv
    gam_np = np.ascontiguousarray(norm_w.reshape(CO, 128).T)
    bet_np = np.ascontiguousarray(norm_b.reshape(CO, 128).T)
    pbt_np = np.ascontiguousarray(pb_eff.astype(np.float32).reshape(CO, 128).T)

    in_maps = []
    for c in range(N_CORES):
        in_maps.append({
            "xs": x[c * BPC:(c + 1) * BPC],
            "wqk": wqk_np, "wpv": wpv_np, "wf": wf_np,
            "gam": gam_np, "bet": bet_np, "pbt": pbt_np,
        })

    res = run_bass_kernel_spmd(nc, in_maps, core_ids=list(range(N_CORES)),
                               trace=TRACE)
    if TRACE:
        kernel.last_exec_time_ns = res.exec_time_ns
        kernel.last_mean_exec_time_ns = res.mean_exec_time_ns
        kernel.last_trace = res.instructions_and_trace
    out = np.concatenate([res.results[c]["out"] for c in range(N_CORES)], axis=0)
    return np.ascontiguousarray(out.reshape(B, C, H, W).astype(np.float32))


# revision 22
# speedup vs baseline: 1.1522x; 1.1522x over previous
"""AttentionBlock (GroupNorm + 1x1-conv self-attention + proj + residual) on 8 TRN2 cores.

Strategy: data-parallel over batch (16 samples -> 2 per core). Per sample, the
whole block runs out of SBUF with two host-side weight fusions that exploit the
bilinearity of attention:
  - logits = (Wq xn)^T (Wk xn) = xn^T (Wq^T Wk) xn, so one fused weight
    Wqk = Wk^T Wq gives z = Wqk xn and logitsT[m,n] = xn[:,m] . z[:,n]
    (the k-bias term is constant along the softmax axis and drops; the q-bias
    term is restored as a per-partition bias in the exp when nonzero).
  - proj(V E) = (Wp Wv) xn E, so Wpv = Wp Wv gives pvT[m,o] = xn[:,m].WpvT[:,o]
    (computed directly transposed by making xn the stationary operand) and
    PU = pvT^T E needs no separate v / attn-out / proj matmuls. The v-bias
    contributes (Wp vb) * denom[n], which after normalization is a constant
    per-channel shift folded into the proj bias.
  This cuts the per-sample matmul count from 272 to 208.
  - GroupNorm(8 groups): per-partition bn_stats on DVE, cross-partition group
    sums via one masked full-K ones-matmul (fp32r; M=1 / partial-K matmuls
    abort on this hw, so mask + full 128-row ones instead).
  - softmax: exp on ScalarE with the 1/sqrt(c) scale folded in (max-subtraction
    skipped: scaled logits are ~N(0,1)), denominator via ones-matmul rows (every
    psum row = denom, no cross-partition broadcast), reciprocal_approx_fast,
    normalization deferred past the (fused) AV+proj matmul into the final
    residual-add pass on VectorE.
  - Data-path matmuls in fp16 (chain error ~1e-4, measured on hw).
"""

import os
import sys

for _p in ("/root/.axon_site", "/root/.axon_site/_ro/trn_rl_repo", "/opt/trn_rl_repo"):
    if os.path.isdir(_p) and _p not in sys.path:
        sys.path.append(_p)

import numpy as np

import concourse.bass as bass
import concourse.tile as tile
from concourse import bacc, mybir
from concourse.bass_utils import run_bass_kernel_spmd

N_CORES = 8
B, C, H, W = 16, 512, 32, 32
HW = H * W                  # 1024 spatial positions
BPC = B // N_CORES          # samples per core
CO = C // 128               # 4 channel tiles
MT = HW // 128              # 8 spatial (m) tiles
NCH = HW // 512             # 2 free-dim chunks of 512
GROUPS = 8
EPS = 1e-5
SCALE = float(C) ** -0.5
N_WARM = 40                 # dummy matmuls bridging the PE through startup

F32 = mybir.dt.float32
F32R = mybir.dt.float32r
F16 = mybir.dt.float16
AF = mybir.ActivationFunctionType

TRACE = False               # test.py sets kernel.TRACE = True for NTFF timing

_CACHE: dict = {}


def _build(flags):
    has_qb, has_pb, has_gamma, has_beta = flags
    nc = bacc.Bacc(trn_type="TRN2", target_bir_lowering=False, debug=False,
                   num_devices=N_CORES)

    xs = nc.dram_tensor("xs", [BPC, C, HW], F32, kind="ExternalInput").ap()
    wqk = nc.dram_tensor("wqk", [C, C], F16, kind="ExternalInput").ap()   # (Wk^T Wq)^T
    wpv = nc.dram_tensor("wpv", [C, C], F16, kind="ExternalInput").ap()   # (Wp Wv)^T
    gam = nc.dram_tensor("gam", [128, CO], F32, kind="ExternalInput").ap()
    bet = nc.dram_tensor("bet", [128, CO], F32, kind="ExternalInput").ap()
    wf = nc.dram_tensor("wf", [C, 1], F16, kind="ExternalInput").ap()     # Wk^T bq
    pbt = nc.dram_tensor("pbt", [128, CO], F32, kind="ExternalInput").ap()
    out = nc.dram_tensor("out", [BPC, C, HW], F32, kind="ExternalOutput").ap()

    with tile.TileContext(nc) as tc:
        with (
            tc.tile_pool(name="wpool", bufs=1) as wpool,
            tc.tile_pool(name="xfp", bufs=2) as xfp,
            tc.tile_pool(name="xnp", bufs=2) as xnp,
            tc.tile_pool(name="zpv", bufs=2) as zpv,
            tc.tile_pool(name="fin", bufs=4) as finp,
            tc.tile_pool(name="hpool", bufs=2) as hpool,
            tc.tile_pool(name="small", bufs=4) as small,
            tc.tile_pool(name="mmps", bufs=(6 if has_qb else 7), space="PSUM") as mmps,
            tc.tile_pool(name="dps", bufs=1, space="PSUM") as dps,
        ):
            # ---- x(s0) first so groupnorm starts ASAP; weights overlap ----
            xs_t = [xs[s].rearrange("(co p) n -> p co n", p=128) for s in range(BPC)]
            xfs = []
            for s in range(BPC):
                xfs.append(xfp.tile([128, CO, HW], F32, tag="xf", name=f"xf{s}"))
            for co in range(CO):
                nc.sync.dma_start(xfs[0][:, co], xs_t[0][:, co])

            onesf = wpool.tile([128, 512], F32, tag="onesf")
            nc.vector.memset(onesf, 1.0)
            ones_r = wpool.tile([128, 128], F32R, tag="onesr")
            nc.vector.tensor_copy(ones_r, onesf[:, 0:128])
            ones16 = wpool.tile([128, 512], F16, tag="ones16")
            nc.vector.tensor_copy(ones16, onesf)

            # PE warmup: harmless matmuls bridge the HAM clock gate through the
            # DMA + groupnorm startup so the first real matmuls run at 2.4 GHz.
            def warmup(n, nm):
                warm_ps = mmps.tile([128, 512], F32, tag="mm", name=f"warm{nm}")
                for i in range(n):
                    nc.tensor.matmul(warm_ps, ones16[:, 0:128], ones16,
                                     start=(i == 0), stop=(i == n - 1))
            warmup(26, "a")

            wqk_sb = wpool.tile([128, CO, C], F16, tag="wqk")
            nc.sync.dma_start(wqk_sb, wqk.rearrange("(ci p) o -> p ci o", p=128))
            wpv_sb = wpool.tile([128, CO, C], F16, tag="wpv")
            nc.sync.dma_start(wpv_sb, wpv.rearrange("(ci p) o -> p ci o", p=128))

            gam_sb = wpool.tile([128, CO], F32, tag="gam")
            nc.sync.dma_start(gam_sb, gam)
            bet_sb = wpool.tile([128, CO], F32, tag="bet")
            nc.sync.dma_start(bet_sb, bet)
            pbt_sb = wpool.tile([128, CO], F32, tag="pbt")
            nc.sync.dma_start(pbt_sb, pbt)
            if has_qb:
                wf_sb = wpool.tile([128, CO, 1], F16, tag="wf")
                nc.sync.dma_start(wf_sb, wf.rearrange("(ci p) o -> p ci o", p=128))

            for s in range(1, BPC):
                for co in range(CO):
                    nc.sync.dma_start(xfs[s][:, co], xs_t[s][:, co])

            eps_sb = wpool.tile([1, 1], F32, tag="eps")
            nc.vector.memset(eps_sb, EPS)
            # per-partition group-half masks: lo = partitions 0-63, hi = 64-127
            mask_lo = wpool.tile([128, 1], F32, tag="mlo")
            nc.vector.memset(mask_lo, 0.0)
            nc.vector.memset(mask_lo[0:64], 1.0)
            mask_hi = wpool.tile([128, 1], F32, tag="mhi")
            nc.vector.tensor_sub(mask_hi, onesf[:, 0:1], mask_lo)

            # ================= group norm (emitted per sample) =========
            xns = [None] * BPC
            sq_scratch = wpool.tile([128, 1024], F16, tag="sqscr")

            def groupnorm(s):
                xf = xfs[s]
                # sp_wide[:, 4co:4co+4] = [m*lo, E2*lo, m*hi, E2*hi]: a single
                # full-K ones-matmul yields every group sum on psum row 0.
                # Even co tiles compute stats on ScalarE (activation accum_out),
                # odd ones on VectorE (bn_stats) - halves the serial stats time.
                sp_wide = small.tile([128, CO * 4], F32R, tag="spw")
                for co in range(CO):
                    sp = small.tile([128, 2], F32, tag="sp")
                    if co % 2 == 0:
                        nc.scalar.activation(out=sq_scratch, in_=xf[:, co],
                                             func=AF.Copy, scale=1.0 / 1024.0,
                                             accum_out=sp[:, 0:1])
                        nc.scalar.activation(out=sq_scratch, in_=xf[:, co],
                                             func=AF.Square, scale=1.0 / 32.0,
                                             accum_out=sp[:, 1:2])
                    else:
                        st = small.tile([128, 2, 6], F32, tag="bnst")
                        for ch in range(2):
                            nc.vector.bn_stats(st[:, ch], xf[:, co, ch * 512:(ch + 1) * 512])
                        mv = small.tile([128, 2], F32, tag="mv")
                        nc.vector.bn_aggr(mv, st)
                        # sp = [mean, var + mean^2] (= [mean, E[x^2]])
                        sq = small.tile([128, 1], F32, tag="sq")
                        nc.vector.tensor_mul(sq, mv[:, 0:1], mv[:, 0:1])
                        nc.vector.tensor_copy(sp[:, 0:1], mv[:, 0:1])
                        nc.vector.tensor_add(sp[:, 1:2], mv[:, 1:2], sq)
                    nc.vector.tensor_scalar_mul(sp_wide[:, 4 * co:4 * co + 2], sp, mask_lo)
                    nc.vector.tensor_scalar_mul(sp_wide[:, 4 * co + 2:4 * co + 4], sp, mask_hi)
                gst = mmps.tile([128, 16], F32, tag="mm", name=f"gst{s}")
                nc.tensor.matmul(gst, ones_r, sp_wide, start=True, stop=True)
                return gst

            def groupnorm_finish(s, gst):
                xf = xfs[s]
                gs = small.tile([1, 16], F32, tag="gs")
                nc.scalar.copy(gs, gst[0:1])
                # pk = [gm(8) | grstd(8)], one tile so one broadcast suffices
                pk = small.tile([1, 16], F32, tag="pk")
                nc.vector.tensor_scalar_mul(pk[:, 0:8], gs[:, 0:16:2], 1.0 / 64.0)
                gvar = small.tile([1, 8], F32, tag="gvar")
                nc.vector.tensor_tensor(gvar, pk[:, 0:8], gs[:, 0:16:2],
                                        mybir.AluOpType.mult)   # M * sum(m) = 64 M^2
                nc.vector.tensor_sub(gvar, gs[:, 1:16:2], gvar)  # sum(E2) - 64 M^2
                # sqrt((sum(E2) - 64 M^2)/64 + eps) = group std
                nc.scalar.activation(out=gvar, in_=gvar, func=AF.Sqrt,
                                     bias=eps_sb, scale=1.0 / 64.0)
                nc.vector.reciprocal(pk[:, 8:16], gvar)
                bc = small.tile([128, 16], F32, tag="bc")
                nc.gpsimd.partition_broadcast(bc, pk)
                # partitions 0-63 take even groups, 64-127 odd (blend by mask:
                # partition_broadcast corrupts base-64 out slices on hw)
                aA = small.tile([128, CO], F32, tag="aA")
                bM = small.tile([128, CO], F32, tag="bM")
                for dst, base in ((bM, 0), (aA, 8)):
                    od = small.tile([128, CO], F32, tag="bcod")
                    nc.vector.tensor_sub(od, bc[:, base + 1:base + 8:2],
                                         bc[:, base + 0:base + 8:2])
                    nc.vector.scalar_tensor_tensor(
                        out=dst, in0=od, scalar=mask_hi,
                        in1=bc[:, base + 0:base + 8:2],
                        op0=mybir.AluOpType.mult, op1=mybir.AluOpType.add)
                if has_gamma:
                    nc.vector.tensor_mul(aA, aA, gam_sb)
                bB = small.tile([128, CO], F32, tag="bB")
                nc.vector.tensor_mul(bB, bM, aA)        # M * A
                if has_beta:
                    nc.vector.tensor_sub(bB, bB, bet_sb)  # M*A - beta
                # xn = x*A - (M*A - beta) = (x - M)*A + beta
                xn = xnp.tile([128, CO, HW], F16, tag="xn", name=f"xn{s}")
                for co in range(CO):
                    nc.vector.tensor_scalar(
                        out=xn[:, co], in0=xf[:, co],
                        scalar1=aA[:, co:co + 1], scalar2=bB[:, co:co + 1],
                        op0=mybir.AluOpType.mult, op1=mybir.AluOpType.subtract)
                xns[s] = xn

            gst0 = groupnorm(0)
            warmup(16, "b")          # keep the PE hot while the stats chain runs
            groupnorm_finish(0, gst0)

            # ================= per-sample compute =================
            for s in range(BPC):
                xf, xn = xfs[s], xns[s]
                # ---- z = Wqk xn  ([c, n], fused q^T k weight) ----
                # ki outer: the moving operand stays fixed across the co MMs
                z_sb = zpv.tile([128, CO, HW], F16, tag="z")
                for nch in range(NCH):
                    psz = [mmps.tile([128, 512], F32, tag="mm", name=f"z{s}_{nch}_{co}")
                           for co in range(CO)]
                    for ki in range(CO):
                        for co in range(CO):
                            nc.tensor.matmul(
                                psz[co], wqk_sb[:, ki, 128 * co:128 * (co + 1)],
                                xn[:, ki, 512 * nch:512 * (nch + 1)],
                                start=(ki == 0), stop=(ki == CO - 1))
                    for co in range(CO):
                        nc.scalar.copy(z_sb[:, co, 512 * nch:512 * (nch + 1)], psz[co])
                if s + 1 < BPC:
                    _gst_next = groupnorm(s + 1)
                # ---- pvT[m, o] (fused proj@v, computed pre-transposed) ----
                pvT = zpv.tile([128, MT, C], F16, tag="pvT")
                if has_qb:
                    fq = zpv.tile([128, MT], F32, tag="fq")
                for mh in range(2):
                    pspv = [mmps.tile([128, 512], F32, tag="mm", name=f"pv{s}_{mh}_{i}")
                            for i in range(4)]
                    for ki in range(CO):
                        for i in range(4):
                            mt = 4 * mh + i
                            nc.tensor.matmul(
                                pspv[i], xn[:, ki, 128 * mt:128 * (mt + 1)],
                                wpv_sb[:, ki, :],
                                start=(ki == 0), stop=(ki == CO - 1))
                    for i in range(4):
                        nc.vector.tensor_copy(pvT[:, 4 * mh + i], pspv[i])
                    if has_qb:
                        # f[m] = (Wk^T bq) . xn[:, m], folded into the exp bias
                        # (4 single-column groups share one psum bank)
                        psf = dps.tile([128, 4], F32, tag="mmf", name=f"f{s}_{mh}")
                        for i in range(4):
                            mt = 4 * mh + i
                            for ki in range(CO):
                                nc.tensor.matmul(
                                    psf[:, i:i + 1], xn[:, ki, 128 * mt:128 * (mt + 1)],
                                    wf_sb[:, ki],
                                    start=(ki == 0), stop=(ki == CO - 1))
                        nc.scalar.activation(out=fq[:, 4 * mh:4 * mh + 4],
                                             in_=psf, func=AF.Copy, scale=SCALE)

                if s + 1 < BPC:
                    groupnorm_finish(s + 1, _gst_next)

                # ---- attention by n-half: logitsT = xn^T z, exp, PU = pvT^T E ----
                out_t = out[s].rearrange("(co p) n -> p co n", p=128)
                for h in range(NCH):
                    hs = slice(512 * h, 512 * (h + 1))
                    e_sb = hpool.tile([128, MT, 512], F16, tag="e")
                    for mh in range(2):
                        psl = [mmps.tile([128, 512], F32, tag="mm", name=f"l{s}_{h}_{mh}_{i}")
                               for i in range(4)]
                        for ki in range(CO):
                            for i in range(4):
                                mt = 4 * mh + i
                                nc.tensor.matmul(
                                    psl[i], xn[:, ki, 128 * mt:128 * (mt + 1)],
                                    z_sb[:, ki, hs],
                                    start=(ki == 0), stop=(ki == CO - 1))
                        for i in range(4):
                            mt = 4 * mh + i
                            if has_qb:
                                nc.scalar.activation(out=e_sb[:, mt], in_=psl[i],
                                                     func=AF.Exp, scale=SCALE,
                                                     bias=fq[:, mt:mt + 1])
                            else:
                                nc.scalar.activation(out=e_sb[:, mt], in_=psl[i],
                                                     func=AF.Exp, scale=SCALE)
                    # denominator: pairwise e-tile adds on DVE (emitted with
                    # the exp stream), then one ones-matmul for the
                    # cross-partition sum, placed early in the PU block so rb
                    # is ready the moment the PU matmuls finish
                    t1 = hpool.tile([128, 4, 512], F16, tag="dt1")
                    t2 = hpool.tile([128, 2, 512], F16, tag="dt2")
                    tS = hpool.tile([128, 512], F16, tag="dtS")
                    for i in range(4):
                        nc.vector.tensor_add(t1[:, i], e_sb[:, 2 * i], e_sb[:, 2 * i + 1])
                        if i % 2 == 1:
                            nc.vector.tensor_add(t2[:, i // 2], t1[:, i - 1], t1[:, i])
                    nc.vector.tensor_add(tS, t2[:, 0], t2[:, 1])
                    # PU = pvT^T E (unnormalized); the moving operand e[mi]
                    # is shared by the 4 matmuls of each step
                    pspu = [mmps.tile([128, 512], F32, tag="mm", name=f"u{s}_{h}_{oo}")
                            for oo in range(CO)]
                    dsum = dps.tile([128, 512], F32, tag="dsum")
                    rb = hpool.tile([128, 512], F32, tag="rb")
                    for mi in range(MT):
                        for oo in range(CO):
                            nc.tensor.matmul(pspu[oo],
                                             pvT[:, mi, 128 * oo:128 * (oo + 1)],
                                             e_sb[:, mi],
                                             start=(mi == 0), stop=(mi == MT - 1))
                        if mi == 2:
                            nc.tensor.matmul(dsum, ones16[:, 0:128], tS,
                                             start=True, stop=True)
                            nc.vector.reciprocal_approx_fast(out=rb, in_=dsum)
                    for oo in range(CO):
                        t = finp.tile([128, 512], F32, tag="fin")
                        nc.vector.tensor_mul(t, pspu[oo], rb)
                        fo = finp.tile([128, 512], F32, tag="fo")
                        if has_pb:
                            nc.vector.scalar_tensor_tensor(
                                out=fo, in0=t,
                                scalar=pbt_sb[:, oo:oo + 1], in1=xf[:, oo, hs],
                                op0=mybir.AluOpType.add, op1=mybir.AluOpType.add)
                        else:
                            nc.vector.tensor_add(fo, t, xf[:, oo, hs])
                        nc.sync.dma_start(out_t[:, oo, hs], fo)

    nc.compile()
    return nc


def kernel(x, norm_w, norm_b, qkv_w, qkv_b, proj_w, proj_b):
    x = np.ascontiguousarray(np.asarray(x, dtype=np.float32).reshape(B, C, HW))
    norm_w = np.asarray(norm_w, dtype=np.float32)
    norm_b = np.asarray(norm_b, dtype=np.float32)
    qkv_w = np.asarray(qkv_w, dtype=np.float32)
    qkv_b = np.asarray(qkv_b, dtype=np.float32)
    proj_w = np.asarray(proj_w, dtype=np.float32)
    proj_b = np.asarray(proj_b, dtype=np.float32)

    flags = (
        bool(qkv_b[0:C].any()),
        True,   # proj-bias path also carries the folded v-bias; keep it on
        bool((norm_w != 1.0).any()), bool(norm_b.any()),
    )
    if flags not in _CACHE:
        _CACHE[flags] = _build(flags)
    nc = _CACHE[flags]

    Wq, Wk, Wv = qkv_w[0:C], qkv_w[C:2 * C], qkv_w[2 * C:]
    bq, bv = qkv_b[0:C], qkv_b[2 * C:]
    # z = (Wk^T Wq) xn ; device wants the transpose of that fused weight
    wqk_np = np.ascontiguousarray((Wq.T @ Wk).astype(np.float16))
    # pvT = xn^T (Wp Wv)^T
    wpv_np = np.ascontiguousarray((proj_w @ Wv).T.astype(np.float16))
    wf_np = np.ascontiguousarray((Wk.T @ bq).reshape(C, 1).astype(np.float16))
    # v-bias: proj(V + bv) = proj(V) + (Wp bv) * denom[n]; after the 1/denom
    # normalization that is a constant per-channel shift -> fold into proj_b
    pb_eff = proj_b + proj_w @ bv
    gam_np = np.ascontiguousarray(norm_w.reshape(CO, 128).T)
    bet_np = np.ascontiguousarray(norm_b.reshape(CO, 128).T)
    pbt_np = np.ascontiguousarray(pb_eff.astype(np.float32).reshape(CO, 128).T)

    in_maps = []
    for c in range(N_CORES):
        in_maps.append({
            "xs": x[c * BPC:(c + 1) * BPC],
            "wqk": wqk_np, "wpv": wpv_np, "wf": wf_np,
            "gam": gam_np, "bet": bet_np, "pbt": pbt_np,
        })

    res = run_bass_kernel_spmd(nc, in_maps, core_ids=list(range(N_CORES)),
                               trace=TRACE)
    if TRACE:
        kernel.last_exec_time_ns = res.exec_time_ns
        kernel.last_mean_exec_time_ns = res.mean_exec_time_ns
        kernel.last_trace = res.instructions_and_trace
    out = np.concatenate([res.results[c]["out"] for c in range(N_CORES)], axis=0)
    return np.ascontiguousarray(out.reshape(B, C, H, W).astype(np.float32))


# revision 24
# speedup vs baseline: 1.1550x; 1.0025x over previous
"""AttentionBlock (GroupNorm + 1x1-conv self-attention + proj + residual) on 8 TRN2 cores.

Strategy: data-parallel over batch (16 samples -> 2 per core). Per sample, the
whole block runs out of SBUF with two host-side weight fusions that exploit the
bilinearity of attention:
  - logits = (Wq xn)^T (Wk xn) = xn^T (Wq^T Wk) xn, so one fused weight
    Wqk = Wk^T Wq gives z = Wqk xn and logitsT[m,n] = xn[:,m] . z[:,n]
    (the k-bias term is constant along the softmax axis and drops; the q-bias
    term is restored as a per-partition bias in the exp when nonzero).
  - proj(V E) = (Wp Wv) xn E, so Wpv = Wp Wv gives pvT[m,o] = xn[:,m].WpvT[:,o]
    (computed directly transposed by making xn the stationary operand) and
    PU = pvT^T E needs no separate v / attn-out / proj matmuls. The v-bias
    contributes (Wp vb) * denom[n], which after normalization is a constant
    per-channel shift folded into the proj bias.
  This cuts the per-sample matmul count from 272 to 208.
  - GroupNorm(8 groups): per-partition bn_stats on DVE, cross-partition group
    sums via one masked full-K ones-matmul (fp32r; M=1 / partial-K matmuls
    abort on this hw, so mask + full 128-row ones instead).
  - softmax: exp on ScalarE with the 1/sqrt(c) scale folded in (max-subtraction
    skipped: scaled logits are ~N(0,1)), denominator via ones-matmul rows (every
    psum row = denom, no cross-partition broadcast), reciprocal_approx_fast,
    normalization deferred past the (fused) AV+proj matmul into the final
    residual-add pass on VectorE.
  - Data-path matmuls in fp16 (chain error ~1e-4, measured on hw).
"""

import os
import sys

for _p in ("/root/.axon_site", "/root/.axon_site/_ro/trn_rl_repo", "/opt/trn_rl_repo"):
    if os.path.isdir(_p) and _p not in sys.path:
        sys.path.append(_p)

import numpy as np

import concourse.bass as bass
import concourse.tile as tile
from concourse import bacc, mybir
from concourse.bass_utils import run_bass_kernel_spmd

N_CORES = 8
B, C, H, W = 16, 512, 32, 32
HW = H * W                  # 1024 spatial positions
BPC = B // N_CORES          # samples per core
CO = C // 128               # 4 channel tiles
MT = HW // 128              # 8 spatial (m) tiles
NCH = HW // 512             # 2 free-dim chunks of 512
GROUPS = 8
EPS = 1e-5
SCALE = float(C) ** -0.5
N_WARM = 40                 # dummy matmuls bridging the PE through startup

F32 = mybir.dt.float32
F32R = mybir.dt.float32r
F16 = mybir.dt.float16
AF = mybir.ActivationFunctionType

TRACE = False               # test.py sets kernel.TRACE = True for NTFF timing

_CACHE: dict = {}


def _build(flags):
    has_qb, has_pb, has_gamma, has_beta = flags
    nc = bacc.Bacc(trn_type="TRN2", target_bir_lowering=False, debug=False,
                   num_devices=N_CORES)

    xs = nc.dram_tensor("xs", [BPC, C, HW], F32, kind="ExternalInput").ap()
    wqk = nc.dram_tensor("wqk", [C, C], F16, kind="ExternalInput").ap()   # (Wk^T Wq)^T
    wpv = nc.dram_tensor("wpv", [C, C], F16, kind="ExternalInput").ap()   # (Wp Wv)^T
    gam = nc.dram_tensor("gam", [128, CO], F32, kind="ExternalInput").ap()
    bet = nc.dram_tensor("bet", [128, CO], F32, kind="ExternalInput").ap()
    wf = nc.dram_tensor("wf", [C, 1], F16, kind="ExternalInput").ap()     # Wk^T bq
    pbt = nc.dram_tensor("pbt", [128, CO], F32, kind="ExternalInput").ap()
    out = nc.dram_tensor("out", [BPC, C, HW], F32, kind="ExternalOutput").ap()

    with tile.TileContext(nc) as tc:
        with (
            tc.tile_pool(name="wpool", bufs=1) as wpool,
            tc.tile_pool(name="xfp", bufs=2) as xfp,
            tc.tile_pool(name="xnp", bufs=2) as xnp,
            tc.tile_pool(name="zpv", bufs=2) as zpv,
            tc.tile_pool(name="fin", bufs=4) as finp,
            tc.tile_pool(name="hpool", bufs=2) as hpool,
            tc.tile_pool(name="small", bufs=4) as small,
            tc.tile_pool(name="mmps", bufs=(6 if has_qb else 7), space="PSUM") as mmps,
            tc.tile_pool(name="dps", bufs=1, space="PSUM") as dps,
        ):
            # ---- x(s0) first so groupnorm starts ASAP; weights overlap ----
            xs_t = [xs[s].rearrange("(co p) n -> p co n", p=128) for s in range(BPC)]
            xfs = []
            for s in range(BPC):
                xfs.append(xfp.tile([128, CO, HW], F32, tag="xf", name=f"xf{s}"))
            for co in range(CO):
                nc.sync.dma_start(xfs[0][:, co], xs_t[0][:, co])

            onesf = wpool.tile([128, 512], F32, tag="onesf")
            nc.vector.memset(onesf, 1.0)
            ones_r = wpool.tile([128, 128], F32R, tag="onesr")
            nc.vector.tensor_copy(ones_r, onesf[:, 0:128])
            ones16 = wpool.tile([128, 512], F16, tag="ones16")
            nc.vector.tensor_copy(ones16, onesf)

            # PE warmup: harmless matmuls bridge the HAM clock gate through the
            # DMA + groupnorm startup so the first real matmuls run at 2.4 GHz.
            def warmup(n, nm):
                warm_ps = mmps.tile([128, 512], F32, tag="mm", name=f"warm{nm}")
                for i in range(n):
                    nc.tensor.matmul(warm_ps, ones16[:, 0:128], ones16,
                                     start=(i == 0), stop=(i == n - 1))
            warmup(26, "a")

            wqk_sb = wpool.tile([128, CO, C], F16, tag="wqk")
            nc.sync.dma_start(wqk_sb, wqk.rearrange("(ci p) o -> p ci o", p=128))
            wpv_sb = wpool.tile([128, CO, C], F16, tag="wpv")
            nc.sync.dma_start(wpv_sb, wpv.rearrange("(ci p) o -> p ci o", p=128))

            gam_sb = wpool.tile([128, CO], F32, tag="gam")
            nc.sync.dma_start(gam_sb, gam)
            bet_sb = wpool.tile([128, CO], F32, tag="bet")
            nc.sync.dma_start(bet_sb, bet)
            pbt_sb = wpool.tile([128, CO], F32, tag="pbt")
            nc.sync.dma_start(pbt_sb, pbt)
            if has_qb:
                wf_sb = wpool.tile([128, CO, 1], F16, tag="wf")
                nc.sync.dma_start(wf_sb, wf.rearrange("(ci p) o -> p ci o", p=128))

            for s in range(1, BPC):
                for co in range(CO):
                    nc.sync.dma_start(xfs[s][:, co], xs_t[s][:, co])

            eps_sb = wpool.tile([1, 1], F32, tag="eps")
            nc.vector.memset(eps_sb, EPS)
            # per-partition group-half masks: lo = partitions 0-63, hi = 64-127
            mask_lo = wpool.tile([128, 1], F32, tag="mlo")
            nc.vector.memset(mask_lo, 0.0)
            nc.vector.memset(mask_lo[0:64], 1.0)
            mask_hi = wpool.tile([128, 1], F32, tag="mhi")
            nc.vector.tensor_sub(mask_hi, onesf[:, 0:1], mask_lo)

            # ================= group norm (emitted per sample) =========
            xns = [None] * BPC
            sq_scratch = wpool.tile([128, 1024], F16, tag="sqscr")

            def groupnorm(s):
                xf = xfs[s]
                # sp_wide[:, 4co:4co+4] = [m*lo, E2*lo, m*hi, E2*hi]: a single
                # full-K ones-matmul yields every group sum on psum row 0.
                # Even co tiles compute stats on ScalarE (activation accum_out),
                # odd ones on VectorE (bn_stats) - halves the serial stats time.
                sp_wide = small.tile([128, CO * 4], F32R, tag="spw")
                for co in range(CO):
                    sp = small.tile([128, 2], F32, tag="sp")
                    if co % 2 == 0:
                        nc.scalar.activation(out=sq_scratch, in_=xf[:, co],
                                             func=AF.Copy, scale=1.0 / 1024.0,
                                             accum_out=sp[:, 0:1])
                        nc.scalar.activation(out=sq_scratch, in_=xf[:, co],
                                             func=AF.Square, scale=1.0 / 32.0,
                                             accum_out=sp[:, 1:2])
                    else:
                        st = small.tile([128, 2, 6], F32, tag="bnst")
                        for ch in range(2):
                            nc.vector.bn_stats(st[:, ch], xf[:, co, ch * 512:(ch + 1) * 512])
                        mv = small.tile([128, 2], F32, tag="mv")
                        nc.vector.bn_aggr(mv, st)
                        # sp = [mean, var + mean^2] (= [mean, E[x^2]])
                        sq = small.tile([128, 1], F32, tag="sq")
                        nc.vector.tensor_mul(sq, mv[:, 0:1], mv[:, 0:1])
                        nc.vector.tensor_copy(sp[:, 0:1], mv[:, 0:1])
                        nc.vector.tensor_add(sp[:, 1:2], mv[:, 1:2], sq)
                    nc.vector.tensor_scalar_mul(sp_wide[:, 4 * co:4 * co + 2], sp, mask_lo)
                    nc.vector.tensor_scalar_mul(sp_wide[:, 4 * co + 2:4 * co + 4], sp, mask_hi)
                gst = mmps.tile([128, 16], F32, tag="mm", name=f"gst{s}")
                nc.tensor.matmul(gst, ones_r, sp_wide, start=True, stop=True)
                return gst

            def groupnorm_finish(s, gst):
                xf = xfs[s]
                gs = small.tile([1, 16], F32, tag="gs")
                nc.scalar.copy(gs, gst[0:1])
                # pk = [gm(8) | grstd(8)], one tile so one broadcast suffices
                pk = small.tile([1, 16], F32, tag="pk")
                nc.vector.tensor_scalar_mul(pk[:, 0:8], gs[:, 0:16:2], 1.0 / 64.0)
                gvar = small.tile([1, 8], F32, tag="gvar")
                nc.vector.tensor_tensor(gvar, pk[:, 0:8], gs[:, 0:16:2],
                                        mybir.AluOpType.mult)   # M * sum(m) = 64 M^2
                nc.vector.tensor_sub(gvar, gs[:, 1:16:2], gvar)  # sum(E2) - 64 M^2
                # sqrt((sum(E2) - 64 M^2)/64 + eps) = group std
                nc.scalar.activation(out=gvar, in_=gvar, func=AF.Sqrt,
                                     bias=eps_sb, scale=1.0 / 64.0)
                nc.vector.reciprocal(pk[:, 8:16], gvar)
                bc = small.tile([128, 16], F32, tag="bc")
                nc.gpsimd.partition_broadcast(bc, pk)
                # partitions 0-63 take even groups, 64-127 odd (blend by mask:
                # partition_broadcast corrupts base-64 out slices on hw)
                aA = small.tile([128, CO], F32, tag="aA")
                bM = small.tile([128, CO], F32, tag="bM")
                for dst, base in ((bM, 0), (aA, 8)):
                    od = small.tile([128, CO], F32, tag="bcod")
                    nc.vector.tensor_sub(od, bc[:, base + 1:base + 8:2],
                                         bc[:, base + 0:base + 8:2])
                    nc.vector.scalar_tensor_tensor(
                        out=dst, in0=od, scalar=mask_hi,
                        in1=bc[:, base + 0:base + 8:2],
                        op0=mybir.AluOpType.mult, op1=mybir.AluOpType.add)
                if has_gamma:
                    nc.vector.tensor_mul(aA, aA, gam_sb)
                bB = small.tile([128, CO], F32, tag="bB")
                nc.vector.tensor_mul(bB, bM, aA)        # M * A
                if has_beta:
                    nc.vector.tensor_sub(bB, bB, bet_sb)  # M*A - beta
                # xn = x*A - (M*A - beta) = (x - M)*A + beta
                xn = xnp.tile([128, CO, HW], F16, tag="xn", name=f"xn{s}")
                for co in range(CO):
                    nc.vector.tensor_scalar(
                        out=xn[:, co], in0=xf[:, co],
                        scalar1=aA[:, co:co + 1], scalar2=bB[:, co:co + 1],
                        op0=mybir.AluOpType.mult, op1=mybir.AluOpType.subtract)
                xns[s] = xn

            gst0 = groupnorm(0)
            warmup(16, "b")          # keep the PE hot while the stats chain runs
            groupnorm_finish(0, gst0)

            # ================= per-sample compute =================
            for s in range(BPC):
                xf, xn = xfs[s], xns[s]
                # ---- z = Wqk xn  ([c, n], fused q^T k weight) ----
                # ki outer: the moving operand stays fixed across the co MMs
                z_sb = zpv.tile([128, CO, HW], F16, tag="z")
                for nch in range(NCH):
                    psz = [mmps.tile([128, 512], F32, tag="mm", name=f"z{s}_{nch}_{co}")
                           for co in range(CO)]
                    for ki in range(CO):
                        for co in range(CO):
                            nc.tensor.matmul(
                                psz[co], wqk_sb[:, ki, 128 * co:128 * (co + 1)],
                                xn[:, ki, 512 * nch:512 * (nch + 1)],
                                start=(ki == 0), stop=(ki == CO - 1))
                    for co in range(CO):
                        nc.scalar.copy(z_sb[:, co, 512 * nch:512 * (nch + 1)], psz[co])
                if s + 1 < BPC:
                    _gst_next = groupnorm(s + 1)
                # ---- pvT[m, o] (fused proj@v, computed pre-transposed) ----
                pvT = zpv.tile([128, MT, C], F16, tag="pvT")
                if has_qb:
                    fq = zpv.tile([128, MT], F32, tag="fq")
                for mh in range(2):
                    pspv = [mmps.tile([128, 512], F32, tag="mm", name=f"pv{s}_{mh}_{i}")
                            for i in range(4)]
                    for ki in range(CO):
                        for i in range(4):
                            mt = 4 * mh + i
                            nc.tensor.matmul(
                                pspv[i], xn[:, ki, 128 * mt:128 * (mt + 1)],
                                wpv_sb[:, ki, :],
                                start=(ki == 0), stop=(ki == CO - 1))
                    for i in range(4):
                        nc.scalar.copy(pvT[:, 4 * mh + i], pspv[i])
                    if has_qb:
                        # f[m] = (Wk^T bq) . xn[:, m], folded into the exp bias
                        # (4 single-column groups share one psum bank)
                        psf = dps.tile([128, 4], F32, tag="mmf", name=f"f{s}_{mh}")
                        for i in range(4):
                            mt = 4 * mh + i
                            for ki in range(CO):
                                nc.tensor.matmul(
                                    psf[:, i:i + 1], xn[:, ki, 128 * mt:128 * (mt + 1)],
                                    wf_sb[:, ki],
                                    start=(ki == 0), stop=(ki == CO - 1))
                        nc.scalar.activation(out=fq[:, 4 * mh:4 * mh + 4],
                                             in_=psf, func=AF.Copy, scale=SCALE)

                if s + 1 < BPC:
                    groupnorm_finish(s + 1, _gst_next)

                # ---- attention by n-half: logitsT = xn^T z, exp, PU = pvT^T E ----
                out_t = out[s].rearrange("(co p) n -> p co n", p=128)
                for h in range(NCH):
                    hs = slice(512 * h, 512 * (h + 1))
                    e_sb = hpool.tile([128, MT, 512], F16, tag="e")
                    for mh in range(2):
                        psl = [mmps.tile([128, 512], F32, tag="mm", name=f"l{s}_{h}_{mh}_{i}")
                               for i in range(4)]
                        for ki in range(CO):
                            for i in range(4):
                                mt = 4 * mh + i
                                nc.tensor.matmul(
                                    psl[i], xn[:, ki, 128 * mt:128 * (mt + 1)],
                                    z_sb[:, ki, hs],
                                    start=(ki == 0), stop=(ki == CO - 1))
                        for i in range(4):
                            mt = 4 * mh + i
                            if has_qb:
                                nc.scalar.activation(out=e_sb[:, mt], in_=psl[i],
                                                     func=AF.Exp, scale=SCALE,
                                                     bias=fq[:, mt:mt + 1])
                            else:
                                nc.scalar.activation(out=e_sb[:, mt], in_=psl[i],
                                                     func=AF.Exp, scale=SCALE)
                    # denominator: pairwise e-tile adds on DVE (emitted with
                    # the exp stream), then one ones-matmul for the
                    # cross-partition sum, placed early in the PU block so rb
                    # is ready the moment the PU matmuls finish
                    t1 = hpool.tile([128, 4, 512], F16, tag="dt1")
                    t2 = hpool.tile([128, 2, 512], F16, tag="dt2")
                    tS = hpool.tile([128, 512], F16, tag="dtS")
                    for i in range(4):
                        nc.vector.tensor_add(t1[:, i], e_sb[:, 2 * i], e_sb[:, 2 * i + 1])
                        if i % 2 == 1:
                            nc.vector.tensor_add(t2[:, i // 2], t1[:, i - 1], t1[:, i])
                    nc.vector.tensor_add(tS, t2[:, 0], t2[:, 1])
                    # PU = pvT^T E (unnormalized); the moving operand e[mi]
                    # is shared by the 4 matmuls of each step
                    pspu = [mmps.tile([128, 512], F32, tag="mm", name=f"u{s}_{h}_{oo}")
                            for oo in range(CO)]
                    dsum = dps.tile([128, 512], F32, tag="dsum")
                    rb = hpool.tile([128, 512], F32, tag="rb")
                    for mi in range(MT):
                        for oo in range(CO):
                            nc.tensor.matmul(pspu[oo],
                                             pvT[:, mi, 128 * oo:128 * (oo + 1)],
                                             e_sb[:, mi],
                                             start=(mi == 0), stop=(mi == MT - 1))
                        if mi == 2:
                            nc.tensor.matmul(dsum, ones16[:, 0:128], tS,
                                             start=True, stop=True)
                            nc.vector.reciprocal_approx_fast(out=rb, in_=dsum)
                    for oo in range(CO):
                        t = finp.tile([128, 512], F32, tag="fin")
                        nc.vector.tensor_mul(t, pspu[oo], rb)
                        fo = finp.tile([128, 512], F32, tag="fo")
                        if has_pb:
                            nc.vector.scalar_tensor_tensor(
                                out=fo, in0=t,
                                scalar=pbt_sb[:, oo:oo + 1], in1=xf[:, oo, hs],
                                op0=mybir.AluOpType.add, op1=mybir.AluOpType.add)
                        else:
                            nc.vector.tensor_add(fo, t, xf[:, oo, hs])
                        nc.sync.dma_start(out_t[:, oo, hs], fo)

    nc.compile()
    return nc


def kernel(x, norm_w, norm_b, qkv_w, qkv_b, proj_w, proj_b):
    x = np.ascontiguousarray(np.asarray(x, dtype=np.float32).reshape(B, C, HW))
    norm_w = np.asarray(norm_w, dtype=np.float32)
    norm_b = np.asarray(norm_b, dtype=np.float32)
    qkv_w = np.asarray(qkv_w, dtype=np.float32)
    qkv_b = np.asarray(qkv_b, dtype=np.float32)
    proj_w = np.asarray(proj_w, dtype=np.float32)
    proj_b = np.asarray(proj_b, dtype=np.float32)

    flags = (
        bool(qkv_b[0:C].any()),
        True,   # proj-bias path also carries the folded v-bias; keep it on
        bool((norm_w != 1.0).any()), bool(norm_b.any()),
    )
    if flags not in _CACHE:
        _CACHE[flags] = _build(flags)
    nc = _CACHE[flags]

    Wq, Wk, Wv = qkv_w[0:C], qkv_w[C:2 * C], qkv_w[2 * C:]
    bq, bv = qkv_b[0:C], qkv_b[2 * C:]
    # z = (Wk^T Wq) xn ; device wants the transpose of that fused weight
    wqk_np = np.ascontiguousarray((Wq.T @ Wk).astype(np.float16))
    # pvT = xn^T (Wp Wv)^T
    wpv_np = np.ascontiguousarray((proj_w @ Wv).T.astype(np.float16))
    wf_np = np.ascontiguousarray((Wk.T @ bq).reshape(C, 1).astype(np.float16))
    # v-bias: proj(V + bv) = proj(V) + (Wp bv) * denom[n]; after the 1/denom
    # normalization that is a constant per-channel shift -> fold into proj_b
    pb_eff = proj_b + proj_w @ bv
    gam_np = np.ascontiguousarray(norm_w.reshape(CO, 128).T)
    bet_np = np.ascontiguousarray(norm_b.reshape(CO, 128).T)
    pbt_np = np.ascontiguousarray(pb_eff.astype(np.float32).reshape(CO, 128).T)

    in_maps = []
    for c in range(N_CORES):
        in_maps.append({
            "xs": x[c * BPC:(c + 1) * BPC],
            "wqk": wqk_np, "wpv": wpv_np, "wf": wf_np,
            "gam": gam_np, "bet": bet_np, "pbt": pbt_np,
        })

    try:
        res = run_bass_kernel_spmd(nc, in_maps, core_ids=list(range(N_CORES)),
                                   trace=TRACE)
    except Exception:
        # a wedged axon terminal fails every execute until reset; reset once
        # and retry before giving up
        try:
            import ctypes
            import jax
            jax.devices()
            lib = ctypes.CDLL("/opt/axon/libaxon_pjrt.so")
            lib.axon_reset.restype = ctypes.c_int64
            lib.axon_reset()
        except Exception:
            pass
        res = run_bass_kernel_spmd(nc, in_maps, core_ids=list(range(N_CORES)),
                                   trace=TRACE)
    if TRACE:
        kernel.last_exec_time_ns = res.exec_time_ns
        kernel.last_mean_exec_time_ns = res.mean_exec_time_ns
        kernel.last_trace = res.instructions_and_trace
    out = np.concatenate([res.results[c]["out"] for c in range(N_CORES)], axis=0)
    return np.ascontiguousarray(out.reshape(B, C, H, W).astype(np.float32))
